# revision 1
# baseline (speedup 1.0000x reference)
"""DeepseekV3 MoE (B=2, S=2048, H=1024, E=16 top-2, I=512, shared IS=1024)
on 8 Trainium2 NeuronCores.

Distribution (expert-parallel, full-I/O contract):
  - Host computes the gate (sigmoid top-2) and dispatches tokens by expert id
    (the "all-to-all" of the sharding hint, done host-side since kernel()
    receives full inputs).
  - Core c runs the SwiGLU MLPs of experts 2c and 2c+1 over their gathered
    tokens (capacity-padded to C columns).
  - The shared expert is split 2-way over its intermediate dim IS=1024:
    cores (2p, 2p+1) each run one I=512 half over tokens [1024p, 1024p+1024);
    the host sums the two partial outputs.  This loads half the shared
    weights per core at the cost of 2x token traffic -- a net byte saving.
  - Host applies the gate combine weights and sums routed + shared.

Device layout: activations stay feature-major (X^T: partition=feature,
free=token) so every matmul uses the weight tile as the stationary operand
and no on-device transposes are needed.  Matmuls run as float32r
(full-rate fp32 mode, 1 cycle/row at moving dim >= 256; measured end-to-end
relative error vs the f32 reference: ~2.5e-4).
"""

import time

import numpy as np

import concourse.bass as bass
import concourse.mybir as mybir
import concourse.tile as tile
from concourse.bass_utils import run_bass_kernel_spmd


# Model dims (hardcoded per the problem spec)
B, S, H = 2, 2048, 1024
E, K = 16, 2
I = 512
IS = 1024
T = B * S
N_CORES = 8
E_LOC = E // N_CORES          # routed experts per core
TSH = T // (N_CORES // 2)     # shared-expert tokens per core pair (1024)
KH = H // 128                 # contraction chunks over H
KI = I // 128                 # contraction chunks over I

F32 = mybir.dt.float32
F32R = mybir.dt.float32r


def _split_sync_waits(nc, maxw=1):
    """This walrus build's setupSyncWait rejects instructions carrying more
    than ~1 semaphore wait.  Hoist excess waits onto same-engine NoOps
    placed immediately before the instruction (same block order => same
    engine program order => identical stall semantics)."""
    uid = 0
    for f in nc.m.functions:
        for bb in f.blocks:
            out = []
            for inst in bb.instructions:
                si = inst.sync_info
                if si is not None and len(si.on_wait) > maxw:
                    waits = list(si.on_wait)
                    for w in waits[:-maxw]:
                        uid += 1
                        out.append(mybir.InstNoOp(
                            name=f"{inst.name}-sw{uid}",
                            opcode="NoOp",
                            engine=inst.engine,
                            ins=[], outs=[],
                            sync_info=mybir.SyncInfo(on_wait=[w], on_update=[]),
                            bass_nofuse=True,
                        ))
                    si.on_wait[:] = waits[-maxw:]
                out.append(inst)
            bb.instructions[:] = out


def _chunks(tok):
    """Split a token count into moving-dim chunks that keep float32r at
    full rate (>=256) and within the fp32 moving-operand max (512)."""
    if tok <= 512:
        return [(0, tok)]
    out = []
    pos = 0
    rem = tok
    while rem > 0:
        w = 512 if rem >= 768 else (rem if rem <= 512 else rem // 2)
        out.append((pos, w))
        pos += w
        rem -= w
    return out


def build_device_program(C, split_waits=True, repeat=1, cfg=None):
    """One SPMD program, identical on every core."""
    nc = bass.Bass()

    xg = nc.declare_dram_parameter("xg", [E_LOC, H, C], F32R, isOutput=False)
    xs = nc.declare_dram_parameter("xs", [H, TSH], F32R, isOutput=False)
    weg = nc.declare_dram_parameter("weg", [E_LOC, H, I], F32R, isOutput=False)
    weu = nc.declare_dram_parameter("weu", [E_LOC, H, I], F32R, isOutput=False)
    wed = nc.declare_dram_parameter("wed", [E_LOC, I, H], F32R, isOutput=False)
    wsg = nc.declare_dram_parameter("wsg", [H, I], F32R, isOutput=False)
    wsu = nc.declare_dram_parameter("wsu", [H, I], F32R, isOutput=False)
    wsd = nc.declare_dram_parameter("wsd", [I, H], F32R, isOutput=False)
    yg = nc.declare_dram_parameter("yg", [E_LOC, H, C], F32, isOutput=True)
    ys = nc.declare_dram_parameter("ys", [H, TSH], F32, isOutput=True)

    if cfg is None:
        cfg = {}
    bufs = dict(xp=16, wgp=16, wup=16, wdp=8, pp=12, gp=6, yp=6,
                psg=2, psu=2, psy=3, store="scalar")
    bufs.update(cfg)

    # Fit the SBUF budget (~206 KB/partition usable here) when C grows
    # beyond 640: xg/p tile slots scale with C, so shrink pool depths in
    # a priority order until the estimate fits.
    slot = max(C * 4, 4096)  # xg tile [128, C] vs xs tile [128, 1024]

    def est():
        return ((bufs["xp"] + bufs["pp"]) * slot
                + (bufs["wgp"] + bufs["wup"]) * 2048 + bufs["wdp"] * 4096
                + bufs["gp"] * 2 * 2048 + bufs["yp"] * 2048)

    shrink = [("xp", 12), ("pp", 8), ("xp", 10), ("pp", 6),
              ("wgp", 12), ("wup", 12), ("wdp", 6), ("gp", 4), ("yp", 4)]
    i = 0
    while est() > 206 * 1024 and i < len(shrink):
        k, v = shrink[i]
        bufs[k] = min(bufs[k], v)
        i += 1

    with tile.TileContext(nc) as tc:
        with (
            tc.tile_pool(name="xp", bufs=bufs["xp"]) as xp,
            tc.tile_pool(name="wgp", bufs=bufs["wgp"]) as wgp,
            tc.tile_pool(name="wup", bufs=bufs["wup"]) as wup,
            tc.tile_pool(name="wdp", bufs=bufs["wdp"]) as wdp,
            tc.tile_pool(name="pp", bufs=bufs["pp"]) as pp,
            tc.tile_pool(name="gp", bufs=bufs["gp"]) as gp,
            tc.tile_pool(name="yp", bufs=bufs["yp"]) as yp,
            tc.tile_pool(name="psg", bufs=bufs["psg"], space="PSUM") as psg,
            tc.tile_pool(name="psu", bufs=bufs["psu"], space="PSUM") as psu,
            tc.tile_pool(name="psy", bufs=bufs["psy"], space="PSUM") as psy,
        ):

            def load_chunks(pool, dram2d, n_k, width):
                tiles = []
                for k in range(n_k):
                    t = pool.tile([128, width], F32R)
                    nc.sync.dma_start(t[:], dram2d[k * 128:(k + 1) * 128, :])
                    tiles.append(t)
                return tiles

            def swiglu_job(segments, wg_dram, wu_dram, wd_dram):
                """One I=512 SwiGLU MLP over a list of token segments
                (x_dram, out_dram, tok); weights are loaded once."""
                wg_t = load_chunks(wgp, wg_dram, KH, I)
                wu_t = load_chunks(wup, wu_dram, KH, I)
                wd_t = load_chunks(wdp, wd_dram, KI, H)
                for (x_dram, out_dram, tok) in segments:
                    chunks = _chunks(tok)
                    x_t = load_chunks(xp, x_dram, KH, tok)
                    p_tiles = []
                    for i_t in range(KI):
                        p = pp.tile([128, tok], F32R)
                        isl = slice(i_t * 128, (i_t + 1) * 128)
                        for (n0, nw) in chunks:
                            nsl = slice(n0, n0 + nw)
                            g_ps = psg.tile([128, nw], F32)
                            for k in range(KH):
                                nc.tensor.matmul(
                                    g_ps[:], wg_t[k][:, isl], x_t[k][:, nsl],
                                    start=(k == 0), stop=(k == KH - 1),
                                )
                            u_ps = psu.tile([128, nw], F32)
                            for k in range(KH):
                                nc.tensor.matmul(
                                    u_ps[:], wu_t[k][:, isl], x_t[k][:, nsl],
                                    start=(k == 0), stop=(k == KH - 1),
                                )
                            # silu(g)*u: sigmoid + two muls (CoreSim lacks Silu)
                            sg = gp.tile([128, nw], F32)
                            nc.scalar.activation(
                                sg[:], g_ps[:],
                                mybir.ActivationFunctionType.Sigmoid,
                            )
                            gs = gp.tile([128, nw], F32)
                            nc.vector.tensor_mul(gs[:], g_ps[:], sg[:])
                            nc.vector.tensor_mul(p[:, nsl], gs[:], u_ps[:])
                        p_tiles.append(p)

                    for h in range(KH):
                        hsl = slice(h * 128, (h + 1) * 128)
                        for (n0, nw) in chunks:
                            nsl = slice(n0, n0 + nw)
                            y_ps = psy.tile([128, nw], F32)
                            for ki in range(KI):
                                nc.tensor.matmul(
                                    y_ps[:], wd_t[ki][:, hsl],
                                    p_tiles[ki][:, nsl],
                                    start=(ki == 0), stop=(ki == KI - 1),
                                )
                            y_sb = yp.tile([128, nw], F32)
                            nc.vector.tensor_copy(y_sb[:], y_ps[:])
                            store_eng = getattr(nc, bufs.get("store", "sync"))
                            store_eng.dma_start(out_dram[hsl, nsl], y_sb[:])

            half = TSH // 2
            shared_segs = ([(xs, ys, TSH)] if bufs.get("shared_seg") == 1 else
                           [(xs[:, :half], ys[:, :half], half),
                            (xs[:, half:], ys[:, half:], half)])
            for _rep in range(repeat):
                jobs = [([(xg[j], yg[j], C)], weg[j], weu[j], wed[j])
                        for j in range(E_LOC)]
                jobs.append((shared_segs, wsg, wsu, wsd))
                if bufs.get("shared_first"):
                    jobs = jobs[-1:] + jobs[:-1]
                for segs, a, b, d in jobs:
                    swiglu_job(segs, a, b, d)

    if split_waits:
        _split_sync_waits(nc)
    return nc


def _route(x2, gate_weight):
    """Replicate the reference gate: sigmoid scores, top-2 (ties -> lower
    index), normalized weights.  float64 internally for stable ranking."""
    logits = x2.astype(np.float64) @ gate_weight.astype(np.float64).T
    scores = 1.0 / (1.0 + np.exp(-logits))
    topk_idx = np.argsort(-scores, axis=1, kind="stable")[:, :K]
    topk_w = np.take_along_axis(scores, topk_idx, axis=1)
    topk_w = topk_w / (topk_w.sum(-1, keepdims=True) + 1e-20)
    return topk_idx.astype(np.int64), topk_w.astype(np.float32)


def kernel(hidden_states, gate_weight, We_gate, We_up, We_down,
           Ws_gate, Ws_up, Ws_down):
    hidden_states = np.asarray(hidden_states, dtype=np.float32)
    gate_weight = np.asarray(gate_weight, dtype=np.float32)
    We_gate = np.asarray(We_gate, dtype=np.float32)
    We_up = np.asarray(We_up, dtype=np.float32)
    We_down = np.asarray(We_down, dtype=np.float32)
    Ws_gate = np.asarray(Ws_gate, dtype=np.float32)
    Ws_up = np.asarray(Ws_up, dtype=np.float32)
    Ws_down = np.asarray(Ws_down, dtype=np.float32)

    x2 = hidden_states.reshape(T, H)
    topk_idx, topk_w = _route(x2, gate_weight)

    # Dispatch: group the T*K (token, slot) assignments by expert.
    assign = topk_idx.ravel()                       # [T*K]
    order = np.argsort(assign, kind="stable")       # slots grouped by expert
    counts = np.bincount(assign, minlength=E)
    starts = np.concatenate([[0], np.cumsum(counts)[:-1]])
    pos = np.empty(T * K, np.int64)                 # slot within its expert
    pos[order] = np.arange(T * K) - starts[assign[order]]

    C = max(640, int(-(-counts.max() // 128)) * 128)  # capacity, mult of 128

    nc = build_device_program(C)

    xT = x2.T  # [H, T] view; column slices below copy what they need
    in_maps = []
    for c in range(N_CORES):
        pair, half = divmod(c, 2)
        xg_np = np.zeros((E_LOC, H, C), np.float32)
        for j in range(E_LOC):
            e = E_LOC * c + j
            toks = order[starts[e]:starts[e] + counts[e]] // K
            xg_np[j, :, :counts[e]] = x2[toks].T
        in_maps.append({
            "xg": xg_np,
            "xs": np.ascontiguousarray(xT[:, TSH * pair:TSH * (pair + 1)]),
            "weg": np.ascontiguousarray(We_gate[E_LOC * c:E_LOC * (c + 1)]),
            "weu": np.ascontiguousarray(We_up[E_LOC * c:E_LOC * (c + 1)]),
            "wed": np.ascontiguousarray(We_down[E_LOC * c:E_LOC * (c + 1)]),
            "wsg": np.ascontiguousarray(Ws_gate[:, I * half:I * (half + 1)]),
            "wsu": np.ascontiguousarray(Ws_up[:, I * half:I * (half + 1)]),
            "wsd": np.ascontiguousarray(Ws_down[I * half:I * (half + 1), :]),
        })

    # The execution stack occasionally reports a transient device error
    # (e.g. NRT_EXEC_UNIT_UNRECOVERABLE through axon) that clears on the
    # next attempt; retry a couple of times before giving up.
    last_exc = None
    for attempt in range(3):
        try:
            res = run_bass_kernel_spmd(
                nc, in_maps, core_ids=list(range(N_CORES)))
            break
        except Exception as exc:  # noqa: BLE001 - deliberate broad retry
            last_exc = exc
            if attempt == 2:
                raise
            time.sleep(5.0)
            nc = build_device_program(C)

    # Gather: per-expert outputs are [H, C] feature-major -> [E, C, H]
    flat_y = np.empty((E, C, H), np.float32)
    ys_all = np.empty((T, H), np.float32)
    for c in range(N_CORES):
        r = res.results[c]
        for j in range(E_LOC):
            flat_y[E_LOC * c + j] = r["yg"][j].T
    for pair in range(N_CORES // 2):
        ysum = res.results[2 * pair]["ys"] + res.results[2 * pair + 1]["ys"]
        ys_all[TSH * pair:TSH * (pair + 1)] = ysum.T

    yr = (topk_w[:, 0:1] * flat_y[topk_idx[:, 0], pos[0::2]]
          + topk_w[:, 1:2] * flat_y[topk_idx[:, 1], pos[1::2]])

    return (yr + ys_all).reshape(B, S, H).astype(np.float32)



# revision 19
# speedup vs baseline: 1.6754x; 1.6754x over previous
"""DeepseekV3 MoE (B=2, S=2048, H=1024, E=16 top-2, I=512, shared IS=1024)
on 8 Trainium2 NeuronCores.

Distribution (expert-parallel, full-I/O contract):
  - Host computes the gate (sigmoid top-2) and dispatches tokens by expert id.
  - Core c runs the SwiGLU MLPs of experts 2c and 2c+1 over their gathered
    tokens (padded to C columns, C = max expert count).
  - The shared expert is split 2-way over its intermediate dim IS=1024:
    cores (2p, 2p+1) each run one I=512 half over tokens [1024p, 1024p+1024);
    the host sums the two partial outputs.
  - Host applies the gate combine weights and sums routed + shared.

Numerics: every matmul runs in fp8e4 (e4m3) DoubleRow mode (256-deep
contraction, 2 rows/cycle) with 3-term hi+lo error compensation:
    A @ B ~= Ah @ Bh + Ah @ Bl + Al @ Bh
where Ah = fp8(A), Al = fp8(A - Ah).  e4m3's narrow exponent range would
flush the residuals of small values to zero, so operands are pre-scaled by
powers of two (x by 16, weights by 64, the SwiGLU intermediate by 16) and
descaled exactly via Activation-engine scale arguments.  Measured end-to-end
relative error vs the f32 reference: ~2.6e-3.

Layout: activations stay feature-major (partition=feature, free=token).
DoubleRow operands are packed host-side as [128, kpairs, 2, cols] so each
matmul slices SBUF tiles directly; each DRAM tensor loads with ONE DMA
(contiguous 4-8KB per-partition rows, full HBM bandwidth).
"""

import time

import ml_dtypes
import numpy as np

import concourse.bass as bass
import concourse.mybir as mybir
import concourse.tile as tile
from concourse.bass_utils import run_bass_kernel_spmd

# Model dims (hardcoded per the problem spec)
B, S, H = 2, 2048, 1024
E, K = 16, 2
I = 512
IS = 1024
T = B * S
N_CORES = 8
E_LOC = E // N_CORES          # routed experts per core
TSH = T // (N_CORES // 2)     # shared-expert tokens per core pair (1024)
KH = H // 128                 # output H blocks
KH2 = H // 256                # DoubleRow k-pairs over H
KI2 = I // 256                # DoubleRow k-pairs over I
NB = 512                      # moving-dim chunk

SX, SW, SP = 16.0, 64.0, 16.0  # fp8 pre-scales: x, weights, swiglu intermediate

F32 = mybir.dt.float32
BF16 = mybir.dt.bfloat16
FP8 = mybir.dt.float8e4
NF8 = ml_dtypes.float8_e4m3
DR = mybir.MatmulPerfMode.DoubleRow
AF = mybir.ActivationFunctionType


def _split_sync_waits(nc, maxw=1):
    """This walrus build's setupSyncWait rejects instructions carrying more
    than ~1 semaphore wait.  Hoist excess waits onto same-engine NoOps
    placed immediately before the instruction (same block order => same
    engine program order => identical stall semantics)."""
    uid = 0
    for f in nc.m.functions:
        for bb in f.blocks:
            out = []
            for inst in bb.instructions:
                si = inst.sync_info
                if si is not None and len(si.on_wait) > maxw:
                    waits = list(si.on_wait)
                    for w in waits[:-maxw]:
                        uid += 1
                        out.append(mybir.InstNoOp(
                            name=f"{inst.name}-sw{uid}",
                            opcode="NoOp",
                            engine=inst.engine,
                            ins=[], outs=[],
                            sync_info=mybir.SyncInfo(on_wait=[w], on_update=[]),
                            bass_nofuse=True,
                        ))
                    si.on_wait[:] = waits[-maxw:]
                out.append(inst)
            bb.instructions[:] = out


def _chunks(tok, nb=NB):
    out, pos = [], 0
    while pos < tok:
        w = min(nb, tok - pos)
        out.append((pos, w))
        pos += w
    return out


def build_device_program(C, split_waits=True, repeat=1, cfg=None):
    """One SPMD program, identical on every core.  C is the per-slot token
    capacity: an int (same for both routed slots) or a tuple (C0, C1)."""
    caps = (C, C) if isinstance(C, int) else tuple(C)
    nc = bass.Bass()
    cfg = dict(cfg or {})

    def par(name, shape, dt=FP8, out=False):
        return nc.declare_dram_parameter(name, shape, dt, isOutput=out)

    xgh = [par(f"xgh{j}", [128, KH2, 2, caps[j]]) for j in range(E_LOC)]
    xgl = [par(f"xgl{j}", [128, KH2, 2, caps[j]]) for j in range(E_LOC)]
    xsh = par("xsh", [128, KH2, 2, TSH])
    xsl = par("xsl", [128, KH2, 2, TSH])
    wegh = par("wegh", [E_LOC, 128, KH2, 2, I])
    wegl = par("wegl", [E_LOC, 128, KH2, 2, I])
    weuh = par("weuh", [E_LOC, 128, KH2, 2, I])
    weul = par("weul", [E_LOC, 128, KH2, 2, I])
    wedh = par("wedh", [E_LOC, 128, KI2, 2, H])
    wedl = par("wedl", [E_LOC, 128, KI2, 2, H])
    wsgh = par("wsgh", [128, KH2, 2, I])
    wsgl = par("wsgl", [128, KH2, 2, I])
    wsuh = par("wsuh", [128, KH2, 2, I])
    wsul = par("wsul", [128, KH2, 2, I])
    wsdh = par("wsdh", [128, KI2, 2, H])
    wsdl = par("wsdl", [128, KI2, 2, H])
    yg = [par(f"yg{j}", [128, KH, caps[j]], BF16, out=True)
          for j in range(E_LOC)]
    ys = par("ys", [128, KH, TSH], BF16, out=True)

    nb = cfg.get("nb", 384)
    bufs = dict(wp=14, wq=16, xq=8, xp=4, pp=4, gp=6, yp=4,
                psg=2, psu=2, psy=4)
    bufs.update(cfg.get("bufs", {}))

    with tile.TileContext(nc) as tc:
        with (
            tc.tile_pool(name="wp", bufs=bufs["wp"]) as wp,
            tc.tile_pool(name="wq", bufs=bufs["wq"]) as wq,
            tc.tile_pool(name="xq", bufs=bufs["xq"]) as xq,
            tc.tile_pool(name="xp", bufs=bufs["xp"]) as xp,
            tc.tile_pool(name="pp", bufs=bufs["pp"]) as pp,
            tc.tile_pool(name="gp", bufs=bufs["gp"]) as gp,
            tc.tile_pool(name="yp", bufs=bufs["yp"]) as yp,
            tc.tile_pool(name="psg", bufs=bufs["psg"], space="PSUM") as psg,
            tc.tile_pool(name="psu", bufs=bufs["psu"], space="PSUM") as psu,
            tc.tile_pool(name="psy", bufs=bufs["psy"], space="PSUM") as psy,
        ):

            def load(pool, shape, dram, kp):
                """Whole-tensor load; returns per-k accessor list."""
                t = pool.tile(shape, FP8)
                nc.sync.dma_start(t[:], dram[:])
                return [(t, k) for k in range(kp)]

            def load_one(pool, dram, k, width, tag):
                """Single k-pair load so the first matmul can start after one
                small transfer instead of the whole tensor."""
                t = pool.tile([128, 1, 2, width], FP8, name=tag)
                nc.sync.dma_start(t[:], dram[:, k:k + 1, :, :])
                return (t, 0)

            def emit_l1(tk):
                """First matmul layer + silu/mul/fp8-split chain for one
                column chunk.  Returns state consumed by emit_l2."""
                (wg_h, wg_l, wu_h, wu_l, wd_h, wd_l) = tk["w"]
                xh_t, xl_t = tk["x"]
                nsl = slice(tk["n0"], tk["n0"] + tk["nw"])
                nw = tk["nw"]
                p8h = [pp.tile([128, 2, nw], FP8, name=f"p8h{k}")
                       for k in range(KI2)]
                p8l = [pp.tile([128, 2, nw], FP8, name=f"p8l{k}")
                       for k in range(KI2)]
                for b in range(I // 128):
                    isl = slice(b * 128, (b + 1) * 128)
                    g_ps = psg.tile([128, nw], F32)
                    u_ps = psu.tile([128, nw], F32)
                    for ps, w_h, w_l in ((g_ps, wg_h, wg_l),
                                         (u_ps, wu_h, wu_l)):
                        mms = (
                            [(w_h[k], xh_t[k]) for k in range(KH2)]
                            + [(w_h[k], xl_t[k]) for k in range(KH2)]
                            + [(w_l[k], xh_t[k]) for k in range(KH2)]
                        )
                        for mi, ((wt, wk), (xt, xk)) in enumerate(mms):
                            nc.tensor.matmul(
                                ps[:], wt[:, wk, :, isl], xt[:, xk, :, nsl],
                                start=(mi == 0), stop=(mi == len(mms) - 1),
                                perf_mode=DR,
                            )
                    silu_g = gp.tile([128, nw], F32)
                    nc.scalar.activation(
                        silu_g[:], g_ps[:], AF.Silu, scale=1.0 / (SX * SW))
                    u_sb = gp.tile([128, nw], F32)
                    nc.scalar.activation(
                        u_sb[:], u_ps[:], AF.Copy, scale=SP / (SX * SW))
                    p32 = gp.tile([128, nw], F32)
                    nc.vector.tensor_mul(p32[:], silu_g[:], u_sb[:])
                    ph = p8h[b // 2][:, b % 2, :]
                    nc.gpsimd.tensor_copy(ph, p32[:])
                    nc.vector.tensor_sub(p8l[b // 2][:, b % 2, :], p32[:], ph)
                tk["p8h"], tk["p8l"] = p8h, p8l

            def emit_l2(tk):
                """Down-projection + descaled bf16 store for one chunk."""
                (wg_h, wg_l, wu_h, wu_l, wd_h, wd_l) = tk["w"]
                p8h, p8l = tk["p8h"], tk["p8l"]
                nsl = slice(tk["n0"], tk["n0"] + tk["nw"])
                nw = tk["nw"]
                y_sb = yp.tile([128, KH, nw], BF16)
                for h in range(KH):
                    hsl = slice(h * 128, (h + 1) * 128)
                    y_ps = psy.tile([128, nw], F32)
                    mms = (
                        [(wd_h[k], p8h[k]) for k in range(KI2)]
                        + [(wd_h[k], p8l[k]) for k in range(KI2)]
                        + [(wd_l[k], p8h[k]) for k in range(KI2)]
                    )
                    for mi, ((wt, wk), pt) in enumerate(mms):
                        nc.tensor.matmul(
                            y_ps[:], wt[:, wk, :, hsl], pt[:],
                            start=(mi == 0), stop=(mi == len(mms) - 1),
                            perf_mode=DR,
                        )
                    if h % 2 == 0:
                        nc.scalar.activation(
                            y_sb[:, h, :], y_ps[:], AF.Copy,
                            scale=1.0 / (SP * SW))
                    else:
                        nc.vector.tensor_scalar_mul(
                            y_sb[:, h, :], y_ps[:], 1.0 / (SP * SW))
                    if tk.get("streamy"):
                        nc.scalar.dma_start(
                            tk["y"][:, h:h + 1, nsl], y_sb[:, h:h + 1, :])
                if not tk.get("streamy"):
                    nc.scalar.dma_start(tk["y"][:, :, nsl], y_sb[:])

            jobs = []
            for j in range(E_LOC):
                jobs.append((
                    [(xgh[j], xgl[j], yg[j], caps[j])],
                    (wegh[j], wegl[j], weuh[j], weul[j], wedh[j], wedl[j]),
                ))
            jobs.append((
                [(xsh, xsl, ys, TSH)],
                (wsgh, wsgl, wsuh, wsul, wsdh, wsdl),
            ))

            for _rep in range(repeat):
                # Flatten all (job, segment, chunk) work into one task list;
                # loads are issued at job granularity, ordered so the first
                # chunk's operands arrive first.
                tasks = []
                for ji, (segments, w_dram) in enumerate(jobs):
                    if ji == 0 and cfg.get("ksplit", False):
                        # Interleave k-granular loads in first-use order so
                        # the PE starts ~0.5MB into the transfer stream.
                        wg_h, wg_l, wu_h, wu_l = [], [], [], []
                        xs_t = [[] for _ in range(2 * len(segments))]
                        for k in range(KH2):
                            wg_h.append(load_one(wq, w_dram[0], k, I, "wq"))
                            for si, (xh_d, xl_d, _, tok) in enumerate(segments):
                                xs_t[2 * si].append(
                                    load_one(xq, xh_d, k, tok, "xq"))
                        for k in range(KH2):
                            for si, (xh_d, xl_d, _, tok) in enumerate(segments):
                                xs_t[2 * si + 1].append(
                                    load_one(xq, xl_d, k, tok, "xq"))
                            wg_l.append(load_one(wq, w_dram[1], k, I, "wq"))
                        for k in range(KH2):
                            wu_h.append(load_one(wq, w_dram[2], k, I, "wq"))
                        for k in range(KH2):
                            wu_l.append(load_one(wq, w_dram[3], k, I, "wq"))
                        x_tiles = [(xs_t[2 * si], xs_t[2 * si + 1])
                                   for si in range(len(segments))]
                    else:
                        wg_h = load(wp, [128, KH2, 2, I], w_dram[0], KH2)
                        x_tiles = []
                        for (xh_d, xl_d, _, tok) in segments:
                            th = load(xp, [128, KH2, 2, tok], xh_d, KH2)
                            tl = load(xp, [128, KH2, 2, tok], xl_d, KH2)
                            x_tiles.append((th, tl))
                        wg_l = load(wp, [128, KH2, 2, I], w_dram[1], KH2)
                        wu_h = load(wp, [128, KH2, 2, I], w_dram[2], KH2)
                        wu_l = load(wp, [128, KH2, 2, I], w_dram[3], KH2)
                    wd_h = load(wp, [128, KI2, 2, H], w_dram[4], KI2)
                    wd_l = load(wp, [128, KI2, 2, H], w_dram[5], KI2)
                    w_tiles = (wg_h, wg_l, wu_h, wu_l, wd_h, wd_l)
                    for xt, (_, _, y_dram, tok) in zip(x_tiles, segments):
                        for (n0, nw) in _chunks(tok, nb):
                            tasks.append(dict(
                                w=w_tiles, x=xt, y=y_dram, n0=n0, nw=nw))

                # Keep job order, but move small remainder chunks to the
                # end: the final task's L2+store tail is then tiny.
                tasks = ([t for t in tasks if t["nw"] >= 128]
                         + [t for t in tasks if t["nw"] < 128])
                if cfg.get("streamy", False):
                    tasks[-1]["streamy"] = True
                # 1-stage software pipeline: PE runs chunk i+1's first layer
                # while chunk i's silu/fp8-split chain drains, then its L2.
                for i, tk in enumerate(tasks):
                    emit_l1(tk)
                    if i > 0:
                        emit_l2(tasks[i - 1])
                emit_l2(tasks[-1])

    if split_waits:
        _split_sync_waits(nc)
    return nc


def _route(x2, gate_weight):
    """Replicate the reference gate: sigmoid scores, top-2 (ties -> lower
    index), normalized weights.  float64 internally for stable ranking."""
    logits = x2.astype(np.float64) @ gate_weight.astype(np.float64).T
    scores = 1.0 / (1.0 + np.exp(-logits))
    topk_idx = np.argsort(-scores, axis=1, kind="stable")[:, :K]
    topk_w = np.take_along_axis(scores, topk_idx, axis=1)
    topk_w = topk_w / (topk_w.sum(-1, keepdims=True) + 1e-20)
    return topk_idx.astype(np.int64), topk_w.astype(np.float32)


def _pairing(counts):
    """Pair heavy experts with light ones: slot-0 gets the 8 largest, slot-1
    the 8 smallest (largest with smallest).  Returns (pair_map, (C0, C1))
    where pair_map[c] = (expert_slot0, expert_slot1)."""
    order = np.argsort(-np.asarray(counts), kind="stable")
    pair_map = [(int(order[i]), int(order[E - 1 - i])) for i in range(N_CORES)]
    c0 = max(int(counts[e0]) for e0, _ in pair_map)
    c1 = max(int(counts[e1]) for _, e1 in pair_map)
    return pair_map, (max(c0, 128), max(c1, 128))


def _capacity(counts):
    return _pairing(counts)[1]


def _split8(a):
    hi = np.asarray(a, np.float32).astype(NF8)
    lo = (np.asarray(a, np.float32) - hi.astype(np.float32)).astype(NF8)
    return hi, lo


def _pack(a2d, kp):
    """[Krows, cols] (contraction-major) -> [128, kp, 2, cols] DoubleRow."""
    krows, cols = a2d.shape
    assert krows == kp * 256
    return np.ascontiguousarray(
        a2d.reshape(kp, 2, 128, cols).transpose(2, 0, 1, 3))


def kernel(hidden_states, gate_weight, We_gate, We_up, We_down,
           Ws_gate, Ws_up, Ws_down):
    hidden_states = np.asarray(hidden_states, dtype=np.float32)
    gate_weight = np.asarray(gate_weight, dtype=np.float32)

    x2 = hidden_states.reshape(T, H)
    topk_idx, topk_w = _route(x2, gate_weight)

    # Dispatch: group the T*K (token, slot) assignments by expert.
    assign = topk_idx.ravel()
    order = np.argsort(assign, kind="stable")
    counts = np.bincount(assign, minlength=E)
    starts = np.concatenate([[0], np.cumsum(counts)[:-1]])
    pos = np.empty(T * K, np.int64)
    pos[order] = np.arange(T * K) - starts[assign[order]]

    pair_map, C = _pairing(counts)
    nc = build_device_program(C)

    # fp8 hi/lo of 16*x for all tokens, once; dispatch gathers fp8 bytes.
    xh8, xl8 = _split8(SX * x2)

    def pack_x(rows_hi, rows_lo, cols):
        # rows: [n, H] fp8 -> pad to cols -> [128, KH2, 2, cols]
        out = []
        for r in (rows_hi, rows_lo):
            a = np.zeros((H, cols), NF8)
            a[:, :r.shape[0]] = r.T
            out.append(_pack(a, KH2))
        return out

    w8 = {}
    for name, w in (("eg", We_gate), ("eu", We_up)):
        hi, lo = _split8(SW * np.asarray(w, np.float32))
        w8[name] = [(_pack(hi[e], KH2), _pack(lo[e], KH2)) for e in range(E)]
    hi, lo = _split8(SW * np.asarray(We_down, np.float32))
    w8["ed"] = [(_pack(hi[e], KI2), _pack(lo[e], KI2)) for e in range(E)]
    sg_h, sg_l = _split8(SW * np.asarray(Ws_gate, np.float32))
    su_h, su_l = _split8(SW * np.asarray(Ws_up, np.float32))
    sd_h, sd_l = _split8(SW * np.asarray(Ws_down, np.float32))

    in_maps = []
    for c in range(N_CORES):
        pair, half = divmod(c, 2)
        isl = slice(I * half, I * (half + 1))
        tsl = slice(TSH * pair, TSH * (pair + 1))
        im = {}
        for j in range(E_LOC):
            e = pair_map[c][j]
            toks = order[starts[e]:starts[e] + counts[e]] // K
            gh, gl = pack_x(xh8[toks], xl8[toks], C[j])
            im[f"xgh{j}"] = gh
            im[f"xgl{j}"] = gl
        sh, sl = pack_x(xh8[tsl], xl8[tsl], TSH)
        im.update({
            "xsh": sh, "xsl": sl,
            "wegh": np.stack([w8["eg"][pair_map[c][j]][0] for j in range(E_LOC)]),
            "wegl": np.stack([w8["eg"][pair_map[c][j]][1] for j in range(E_LOC)]),
            "weuh": np.stack([w8["eu"][pair_map[c][j]][0] for j in range(E_LOC)]),
            "weul": np.stack([w8["eu"][pair_map[c][j]][1] for j in range(E_LOC)]),
            "wedh": np.stack([w8["ed"][pair_map[c][j]][0] for j in range(E_LOC)]),
            "wedl": np.stack([w8["ed"][pair_map[c][j]][1] for j in range(E_LOC)]),
            "wsgh": _pack(sg_h[:, isl], KH2), "wsgl": _pack(sg_l[:, isl], KH2),
            "wsuh": _pack(su_h[:, isl], KH2), "wsul": _pack(su_l[:, isl], KH2),
            "wsdh": _pack(sd_h[isl, :], KI2), "wsdl": _pack(sd_l[isl, :], KI2),
        })
        in_maps.append(im)

    # The execution stack occasionally reports a transient device error
    # (e.g. NRT_EXEC_UNIT_UNRECOVERABLE through axon) that clears on a later
    # attempt; retry with backoff before giving up.
    n_attempts = 5
    for attempt in range(n_attempts):
        try:
            res = run_bass_kernel_spmd(
                nc, in_maps, core_ids=list(range(N_CORES)))
            break
        except Exception:  # noqa: BLE001 - deliberate broad retry
            if attempt == n_attempts - 1:
                raise
            time.sleep(3.0 * (2 ** attempt))
            nc = build_device_program(C)

    # Gather: yg [E_LOC, 128, KH, C] bf16 -> flat_y [E, C, H]
    flat_y = np.empty((E, max(C), H), np.float32)
    ys_all = np.empty((T, H), np.float32)
    for c in range(N_CORES):
        r = res.results[c]
        for j in range(E_LOC):
            ygc = np.asarray(r[f"yg{j}"]).astype(np.float32)
            flat_y[pair_map[c][j], :C[j]] = (
                ygc.transpose(2, 1, 0).reshape(C[j], H))
    for pair in range(N_CORES // 2):
        y0 = np.asarray(res.results[2 * pair]["ys"]).astype(np.float32)
        y1 = np.asarray(res.results[2 * pair + 1]["ys"]).astype(np.float32)
        ysum = y0 + y1
        ys_all[TSH * pair:TSH * (pair + 1)] = (
            ysum.transpose(2, 1, 0).reshape(TSH, H))

    yr = (topk_w[:, 0:1] * flat_y[topk_idx[:, 0], pos[0::2]]
          + topk_w[:, 1:2] * flat_y[topk_idx[:, 1], pos[1::2]])

    return (yr + ys_all).reshape(B, S, H).astype(np.float32)
